# revision 15
# baseline (speedup 1.0000x reference)
"""Distributed Trainium2 Bass kernel for llama-style GQA attention block.

B=2, S=2048, D=4096, NH=32, NKV=8, HD=128.  8 NeuronCores, tensor-parallel
over heads (4 q heads + 1 kv head per core), AllToAll to row-sharded layout
before the output projection (avoids the 67MB AllReduce).

All activations live transposed ([feature, seq]) so no on-chip transposes are
needed anywhere except V (computed K-style as V^T then PE-transposed per
128x128 tile): scores are computed as S^T = K-tiles @ Q^T, softmax
normalization is deferred past the PV matmul, and the per-column sums are
reduced with DVE/GpSimd adds plus a single ones-vector matmul on the PE.

Phase 2 is software-pipelined: scores chunks ([128,1024] PSUM, double
buffered) overlap the ScalarE exp of the previous chunk, whose PV matmuls
trail one chunk behind, so the PE never waits on the activation engine.

RoPE is applied in "block" form: the host permutes wq/wk rows within each head
(even pair-components first, odd second) so the on-chip rotate-half is a
partition half-swap (stream_shuffle) instead of a stride-2 interleave.
"""

import sys
import math
import numpy as np

sys.path.insert(0, "/opt/trn_rl_repo")

import ml_dtypes

from concourse import bacc, tile
import concourse.bass as bass
import concourse.mybir as mybir
from concourse.bass_utils import run_bass_kernel_spmd

B, S, D = 2, 2048, 4096
NH, NKV, HD = 32, 8, 128
BS = B * S
NC = 8
NHL = NH // NC          # 4 local q heads
ROWS = BS // NC         # 512 output rows per core
NSC = 8                 # seq chunks of 512 (global rows)
NDT = 32                # D / 128 contraction tiles
KTB = S // HD           # 16 k-tiles per batch
QBB = 4                 # 512-wide q blocks per batch
SCALE = 1.0 / math.sqrt(HD)

F32 = mybir.dt.float32
BF16 = mybir.dt.bfloat16
bf16 = ml_dtypes.bfloat16

# half-swap of the 128 partitions: 32 groups of 4, rotate by 16 groups
SWAP_MASK = [(i + 16) % 32 for i in range(32)]

_CACHE = {}


def _build(mode: str):
    """mode: 'none' (no mask work), 'causal' (skip + binary diag), 'general'."""
    nc = bacc.Bacc("TRN2", target_bir_lowering=False, debug=False, num_devices=NC)

    xT = nc.dram_tensor("xT", [NSC * NDT * 128, 512], BF16, kind="ExternalInput")
    wqs = nc.dram_tensor("wqs", [128, NDT * 512], BF16, kind="ExternalInput")
    wks = nc.dram_tensor("wks", [128, NDT * 128], BF16, kind="ExternalInput")
    wvs = nc.dram_tensor("wvs", [128, NDT * 128], BF16, kind="ExternalInput")
    coss = nc.dram_tensor("coss", [128, S], F32, kind="ExternalInput")
    sins = nc.dram_tensor("sins", [128, S], F32, kind="ExternalInput")
    ident = nc.dram_tensor("ident", [128, 128], BF16, kind="ExternalInput")
    if mode == "causal":
        bins = nc.dram_tensor("bins", [128, 4 * 512], BF16, kind="ExternalInput")
    elif mode == "general":
        maskT = nc.dram_tensor("maskT", [S, S], BF16, kind="ExternalInput")
    wos = nc.dram_tensor("wos", [8 * NDT * 128, 512], BF16, kind="ExternalInput")
    out = nc.dram_tensor("out", [ROWS, D], F32, kind="ExternalOutput")

    with tile.TileContext(nc) as tc:
        with (
            tc.tile_pool(name="persist", bufs=1) as persist,
            tc.tile_pool(name="dram", bufs=1, space="DRAM") as dram,
            tc.tile_pool(name="wo_in", bufs=16) as wop,
        ):
            # persistent SBUF tensors
            qt_sb = [persist.tile([128, BS], BF16, tag=f"qt{h}", name=f"qt{h}") for h in range(NHL)]
            kt_sb = persist.tile([128, BS], BF16, tag="kt")
            v_sb = persist.tile([128, BS], BF16, tag="v")
            ones_bf = persist.tile([128, 1], BF16, tag="ones_bf")
            ones_row = persist.tile([1, 128], BF16, tag="ones_row")
            ident_sb = persist.tile([128, 128], BF16, tag="ident")
            nc.vector.memset(ones_bf[:, :], 1.0)
            nc.vector.memset(ones_row[:, :], 1.0)
            nc.sync.dma_start(out=ident_sb[:, :], in_=ident[:, :])

            a2a_in_h = [dram.tile([NC * 128, 512], BF16, name=f"a2ain{h}")
                        for h in range(NHL)]
            a2a_out_h = [dram.tile([NC * 128, 512], BF16, name=f"a2aout{h}")
                         for h in range(NHL)]

            # ---------------- Phase 1: QKV projections + RoPE ----------------
            with (
                tc.tile_pool(name="w1", bufs=1) as w1,
                tc.tile_pool(name="xin", bufs=36) as xin,
                tc.tile_pool(name="ppsum", bufs=3, space="PSUM") as ppsum,
                tc.tile_pool(name="tpsum", bufs=2, space="PSUM") as tpsum,
                tc.tile_pool(name="rope", bufs=4) as rope,
                tc.tile_pool(name="vtsb", bufs=2) as vtsb,
            ):
                wq_sb = w1.tile([128, NDT * 512], BF16, tag="wq")
                wk_sb = w1.tile([128, NDT * 128], BF16, tag="wk")
                wv_sb = w1.tile([128, NDT * 128], BF16, tag="wv")
                cos_sb = w1.tile([128, S], F32, tag="cos")
                sin_sb = w1.tile([128, S], F32, tag="sin")
                # interleave the first seq-chunk's x tiles with the chunked
                # wq loads so the first matmuls start within a few us (a
                # weights-first order queues 8.5MB of DMA ahead of x)
                first_x = {}
                for c in range(8):
                    nc.sync.dma_start(
                        out=wq_sb[:, c * 2048:(c + 1) * 2048],
                        in_=wqs[:, c * 2048:(c + 1) * 2048],
                    )
                    for dt in range(4 * c, 4 * c + 4):
                        xt = xin.tile([128, 512], BF16, tag="xt", name=f"xt{dt}")
                        nc.sync.dma_start(out=xt[:, :], in_=xT[dt * 128:(dt + 1) * 128, :])
                        first_x[dt] = xt
                nc.sync.dma_start(out=cos_sb[:, :], in_=coss[:, :])
                nc.sync.dma_start(out=sin_sb[:, :], in_=sins[:, :])
                nc.sync.dma_start(out=wk_sb[:, :], in_=wks[:, :])
                nc.sync.dma_start(out=wv_sb[:, :], in_=wvs[:, :])

                for sc in range(NSC):
                    xts = []
                    for dt in range(NDT):
                        if sc == 0:
                            xts.append(first_x[dt])
                            continue
                        xt = xin.tile([128, 512], BF16, tag="xt", name=f"xt{dt}")
                        g = sc * NDT + dt
                        nc.sync.dma_start(
                            out=xt[:, :], in_=xT[g * 128:(g + 1) * 128, :]
                        )
                        xts.append(xt)
                    pos = (sc % 4) * 512

                    def rope_drain(src, dst):
                        rot = rope.tile([128, 512], F32, tag="rot", name="rot")
                        t1 = rope.tile([128, 512], F32, tag="t1", name="t1")
                        nc.vector.stream_shuffle(
                            out=rot[:, :], in_=src[:, :], mask=SWAP_MASK
                        )
                        nc.vector.tensor_mul(
                            out=t1[:, :], in0=src[:, :],
                            in1=cos_sb[:, pos:pos + 512],
                        )
                        nc.vector.tensor_mul(
                            out=rot[:, :], in0=rot[:, :],
                            in1=sin_sb[:, pos:pos + 512],
                        )
                        nc.vector.tensor_add(
                            out=dst[:, sc * 512:(sc + 1) * 512],
                            in0=t1[:, :], in1=rot[:, :],
                        )

                    # each accumulation group gets its own psum tile, groups
                    # run back-to-back (interleaved groups on one tile break
                    # PSUM has_written semantics)
                    for h in range(NHL):
                        pq = ppsum.tile([128, 512], F32, tag="pp", name="pq")
                        for dt in range(NDT):
                            nc.tensor.matmul(
                                out=pq[:, :],
                                lhsT=wq_sb[:, dt * 512 + h * 128:dt * 512 + (h + 1) * 128],
                                rhs=xts[dt][:, :], start=dt == 0, stop=dt == NDT - 1,
                            )
                        rope_drain(pq, qt_sb[h])
                    pk = ppsum.tile([128, 512], F32, tag="pp", name="pk")
                    for dt in range(NDT):
                        nc.tensor.matmul(
                            out=pk[:, :],
                            lhsT=wk_sb[:, dt * 128:(dt + 1) * 128],
                            rhs=xts[dt][:, :], start=dt == 0, stop=dt == NDT - 1,
                        )
                    rope_drain(pk, kt_sb)
                    # V^T K-style (N=512 streams), then PE-transpose per tile
                    pv = ppsum.tile([128, 512], F32, tag="pp", name="pv")
                    for dt in range(NDT):
                        nc.tensor.matmul(
                            out=pv[:, :],
                            lhsT=wv_sb[:, dt * 128:(dt + 1) * 128],
                            rhs=xts[dt][:, :], start=dt == 0, stop=dt == NDT - 1,
                        )
                    vt = vtsb.tile([128, 512], BF16, tag="vt")
                    nc.vector.tensor_copy(out=vt[:, :], in_=pv[:, :])
                    for st in range(4):
                        tp = tpsum.tile([128, 128], BF16, tag="tp", name="tp")
                        nc.tensor.transpose(
                            tp[:, :], vt[:, st * 128:(st + 1) * 128], ident_sb[:, :]
                        )
                        nc.vector.tensor_copy(
                            out=v_sb[:, (sc * 4 + st) * 128:(sc * 4 + st + 1) * 128],
                            in_=tp[:, :],
                        )

            # ---------------- Phase 2: attention ----------------
            with (
                tc.tile_pool(name="p2sb", bufs=1) as p2sb,
                tc.tile_pool(name="probs", bufs=2) as probsp,
                tc.tile_pool(name="sacc", bufs=2) as saccp,
                tc.tile_pool(name="spsum", bufs=2, space="PSUM") as spsum,
                tc.tile_pool(name="otpsum", bufs=2, space="PSUM") as otpsum,
                tc.tile_pool(name="smpsum", bufs=1, space="PSUM") as smpsum,
                tc.tile_pool(name="bcpsum", bufs=1, space="PSUM") as bcpsum,
                tc.tile_pool(name="aosb", bufs=4) as aosb,
                tc.tile_pool(name="msksb", bufs=4) as msksb,
            ):
                if mode == "causal":
                    bin_sb = p2sb.tile([128, 4 * 512], BF16, tag="bin")
                    nc.sync.dma_start(out=bin_sb[:, :], in_=bins[:, :])
                    # the diag-skip exp leaves stale data in the masked-out
                    # probs columns (zeroed by the mask multiply) — memset the
                    # two pool slots once so the first uses can't see NaNs
                    for _ in range(2):
                        pz = probsp.tile([128, KTB * 512], BF16, tag="probs")
                        nc.vector.memset(pz[:, :], 0.0)
                for h in range(NHL):
                    pending = []
                    for b in range(B):
                        for qb in range(QBB):
                            nkt = 4 * qb + 4 if mode == "causal" else KTB
                            nch = nkt // 2
                            qs = qt_sb[h][:, b * S + qb * 512:b * S + (qb + 1) * 512]
                            probs = probsp.tile([128, KTB * 512], BF16, tag="probs")
                            ot = otpsum.tile([128, 512], F32, tag="ot")
                            st = saccp.tile([128, 8 * 512], BF16, tag="st", name="st")

                            def scores(ch):
                                # 2 one-shot matmuls into the 2 bank-aligned
                                # halves of one [128, 1024] psum, one exp
                                ps = spsum.tile([128, 1024], F32, tag="ps", name="ps")
                                for m in range(2):
                                    kt = 2 * ch + m
                                    nc.tensor.matmul(
                                        out=ps[:, m * 512:(m + 1) * 512],
                                        lhsT=kt_sb[:, (b * KTB + kt) * 128:(b * KTB + kt + 1) * 128],
                                        rhs=qs, start=True, stop=True,
                                    )
                                    if mode == "general":
                                        mt = msksb.tile([128, 512], BF16, tag="mt")
                                        nc.sync.dma_start(
                                            out=mt[:, :],
                                            in_=maskT[kt * 128:(kt + 1) * 128,
                                                      qb * 512:(qb + 1) * 512],
                                        )
                                        nc.vector.tensor_add(
                                            out=ps[:, m * 512:(m + 1) * 512],
                                            in0=ps[:, m * 512:(m + 1) * 512],
                                            in1=mt[:, :],
                                        )
                                pslice = probs[:, ch * 1024:(ch + 1) * 1024]
                                if mode == "causal" and ch >= nch - 2:
                                    # diagonal block: exp only at-or-below the
                                    # 128-tile diagonal; the mask multiply
                                    # zeroes the rest (incl. stale columns)
                                    for m in range(2):
                                        off = ((ch - (nch - 2)) * 2 + m) * 128
                                        nc.scalar.activation(
                                            probs[:, ch * 1024 + m * 512 + off:
                                                  ch * 1024 + (m + 1) * 512],
                                            ps[:, m * 512 + off:(m + 1) * 512],
                                            mybir.ActivationFunctionType.Exp,
                                            bias=0.0, scale=SCALE,
                                        )
                                    boff = (ch - (nch - 2)) * 1024
                                    nc.vector.tensor_mul(
                                        out=pslice, in0=pslice,
                                        in1=bin_sb[:, boff:boff + 1024],
                                    )
                                else:
                                    nc.scalar.activation(
                                        pslice, ps[:, :],
                                        mybir.ActivationFunctionType.Exp,
                                        bias=0.0, scale=SCALE,
                                    )

                            def post(ch):
                                # PV for the 2 tiles of chunk ch
                                for m in range(2):
                                    kt = 2 * ch + m
                                    nc.tensor.matmul(
                                        out=ot[:, :],
                                        lhsT=v_sb[:, (b * KTB + kt) * 128:(b * KTB + kt + 1) * 128],
                                        rhs=probs[:, kt * 512:(kt + 1) * 512],
                                        start=kt == 0, stop=kt == nkt - 1,
                                    )

                            def tree(nkt=nkt, probs=probs, st=st):
                                # pairwise-halving softmax-sum tree on GpSimd
                                # (wide bf16 adds, off the DVE critical path)
                                cols = nkt * 512
                                if nkt == 12:
                                    nc.gpsimd.tensor_add(
                                        out=st[:, 0:2048], in0=probs[:, 0:2048],
                                        in1=probs[:, 2048:4096])
                                    nc.gpsimd.tensor_add(
                                        out=st[:, 0:2048], in0=st[:, 0:2048],
                                        in1=probs[:, 4096:6144])
                                    w = 2048
                                else:
                                    w = cols // 2
                                    nc.gpsimd.tensor_add(
                                        out=st[:, 0:w], in0=probs[:, 0:w],
                                        in1=probs[:, w:cols])
                                while w > 512:
                                    hw = w // 2
                                    nc.gpsimd.tensor_add(
                                        out=st[:, 0:hw], in0=st[:, 0:hw],
                                        in1=st[:, hw:w])
                                    w = hw

                            state = {}

                            def tail1(st=st, state=state):
                                # denominators: cross-partition ones-reduction
                                # of the tree sum on the PE + fast reciprocal
                                sm = smpsum.tile([1, 512], F32, tag="sm")
                                nc.tensor.matmul(
                                    out=sm[:, :], lhsT=ones_bf[:, :],
                                    rhs=st[:, 0:512],
                                    start=True, stop=True,
                                )
                                rec32 = aosb.tile([1, 512], F32, tag="rec32")
                                nc.vector.reciprocal_approx_fast(
                                    out=rec32[:, :], in_=sm[:, :]
                                )
                                rec = aosb.tile([1, 512], BF16, tag="rec")
                                with nc.allow_low_precision(reason="softmax recip bf16"):
                                    nc.vector.tensor_copy(out=rec[:, :], in_=rec32[:, :])
                                state["rec"] = rec

                            def tail2(h=h, b=b, qb=qb, ot=ot, state=state):
                                # broadcast 1/denom to 128 partitions and
                                # normalize the PV accumulator
                                bc = bcpsum.tile([128, 512], F32, tag="bc")
                                nc.tensor.matmul(
                                    out=bc[:, :], lhsT=ones_row[:, :],
                                    rhs=state["rec"][:, :],
                                    start=True, stop=True,
                                )
                                bc_sb = aosb.tile([128, 512], F32, tag="bc_sb")
                                nc.vector.tensor_copy(out=bc_sb[:, :], in_=bc[:, :])
                                ao = aosb.tile([128, 512], BF16, tag="ao")
                                nc.vector.tensor_mul(
                                    out=ao[:, :], in0=ot[:, :], in1=bc_sb[:, :]
                                )
                                j = b * 4 + qb
                                nc.sync.dma_start(
                                    out=a2a_in_h[h][j * 128:(j + 1) * 128, :],
                                    in_=ao[:, :],
                                )

                            # the previous block's tail stages fire after this
                            # block's scores chunks so their sm/bc matmuls
                            # never stall the PE behind the DVE reductions
                            for ch in range(nch):
                                scores(ch)
                                if pending:
                                    pending.pop(0)()
                                if ch > 0:
                                    post(ch - 1)
                            post(nch - 1)
                            tree()
                            pending += [tail1, tail2]
                    for fn in pending:
                        fn()
                    pending = []
                    # per-head AllToAll: overlaps with the next head's compute
                    nc.gpsimd.collective_compute(
                        "AllToAll", mybir.AluOpType.bypass,
                        ins=[a2a_in_h[h].opt()], outs=[a2a_out_h[h].opt()],
                        replica_groups=[list(range(NC))],
                    )

            # ---------------- Phase 3: output projection ----------
            with (
                tc.tile_pool(name="attsb", bufs=1) as attp,
                tc.tile_pool(name="ypsum", bufs=8, space="PSUM") as ypsum,
                tc.tile_pool(name="ysb", bufs=4) as ysbp,
            ):
                att_sb = attp.tile([128, NDT * 512], BF16, tag="att")
                # h-major order: tiles for head h usable right after A2A #h
                t_order = [i * 4 + h for h in range(NHL) for i in range(NC)]
                for t in t_order:
                    i, h = t // 4, t % 4
                    nc.sync.dma_start(
                        out=att_sb[:, t * 512:(t + 1) * 512],
                        in_=a2a_out_h[h][i * 128:(i + 1) * 128, :],
                    )

                yps_dc = {}

                def wo_mms(dc, ts, n0):
                    yps = yps_dc[dc]
                    for n, t in enumerate(ts):
                        wot = wop.tile([128, 512], BF16, tag="wot")
                        g = dc * NDT + t
                        nc.sync.dma_start(
                            out=wot[:, :], in_=wos[g * 128:(g + 1) * 128, :]
                        )
                        for st in range(4):
                            nc.tensor.matmul(
                                out=yps[st][:, :],
                                lhsT=att_sb[:, t * 512 + st * 128:t * 512 + (st + 1) * 128],
                                rhs=wot[:, :],
                                start=n0 + n == 0, stop=n0 + n == NDT - 1,
                            )

                def drain(dc):
                    for st in range(4):
                        ysb = ysbp.tile([128, 512], F32, tag="ysb")
                        nc.vector.tensor_copy(out=ysb[:, :], in_=yps_dc[dc][st][:, :])
                        nc.sync.dma_start(
                            out=out[st * 128:(st + 1) * 128, dc * 512:(dc + 1) * 512],
                            in_=ysb[:, :],
                        )

                # heads 0-2 of dc before head 3 of dc-1: the final A2A hides
                # behind two dc passes worth of head-0-2 accumulation
                for dc in range(8):
                    yps_dc[dc] = [
                        ypsum.tile([128, 512], F32, tag="yp", name=f"yp{_s}")
                        for _s in range(4)
                    ]
                    wo_mms(dc, t_order[:24], 0)
                    if dc >= 1:
                        wo_mms(dc - 1, t_order[24:], 24)
                        drain(dc - 1)
                wo_mms(7, t_order[24:], 24)
                drain(7)
    nc.compile()
    return nc


# within each head: 4 windows of 32 partitions = [16 re-pairs | 16 im-pairs],
# so the rotate-half is stream_shuffle's per-32-window rotation by 16.
_PERM_IDX = np.array(
    [
        2 * (w * 16 + (j if j < 16 else j - 16)) + (0 if j < 16 else 1)
        for w in range(4)
        for j in range(32)
    ]
)
_PI = np.array([w * 16 + (j if j < 16 else j - 16) for w in range(4) for j in range(32)])
_SGN = np.array(
    [(-1.0 if j < 16 else 1.0) for w in range(4) for j in range(32)], np.float32
)


def _perm_block(w):
    o = w.reshape(-1, HD, D)
    return o[:, _PERM_IDX, :].reshape(-1, D)


def _stage(x, wq, wk, wv, wo, freqs_cos, freqs_sin, mask):
    """Returns (mode, shared dict, per-core dicts)."""
    causal = np.where(np.triu(np.ones((S, S), dtype=bool), k=1), -1e9, 0.0).astype(
        np.float32
    )
    if not mask.any():
        mode = "none"
    elif np.array_equal(mask, causal):
        mode = "causal"
    else:
        mode = "general"

    xT = np.ascontiguousarray(x.reshape(BS, D).T)  # [D, BS]
    x_st = (
        xT.reshape(NDT, 128, NSC, 512).transpose(2, 0, 1, 3).reshape(NSC * NDT * 128, 512)
    ).astype(bf16)
    woT = np.ascontiguousarray(wo.T)  # [hd, Dout]
    wo_st = (
        woT.reshape(NDT, 128, 8, 512).transpose(2, 0, 1, 3).reshape(8 * NDT * 128, 512)
    ).astype(bf16)
    cosT = freqs_cos.T.astype(np.float32)  # [64, S]
    sinT = freqs_sin.T.astype(np.float32)
    cos_st = np.ascontiguousarray(cosT[_PI, :])
    sin_st = np.ascontiguousarray(sinT[_PI, :] * _SGN[:, None])

    shared = {"xT": x_st, "coss": cos_st, "sins": sin_st, "wos": wo_st,
              "ident": np.eye(128, dtype=bf16)}
    if mode == "causal":
        bin_diag = np.triu(np.ones((512, 512), np.float32)).astype(bf16)
        shared["bins"] = np.ascontiguousarray(
            bin_diag.reshape(4, 128, 512).transpose(1, 0, 2).reshape(128, 2048)
        )
    elif mode == "general":
        shared["maskT"] = np.ascontiguousarray(mask.T * math.sqrt(HD)).astype(bf16)

    per_core = []
    for c in range(NC):
        wq_c = _perm_block(wq[c * 512:(c + 1) * 512]).T  # [D, 512]
        wk_c = _perm_block(wk[c * 128:(c + 1) * 128]).T  # [D, 128]
        wv_c = wv[c * 128:(c + 1) * 128].T               # [D, 128]
        wq_st = wq_c.reshape(NDT, 128, 512).transpose(1, 0, 2).reshape(128, NDT * 512)
        wk_st = wk_c.reshape(NDT, 128, 128).transpose(1, 0, 2).reshape(128, NDT * 128)
        wv_st = wv_c.reshape(NDT, 128, 128).transpose(1, 0, 2).reshape(128, NDT * 128)
        per_core.append(
            {
                "wqs": np.ascontiguousarray(wq_st).astype(bf16),
                "wks": np.ascontiguousarray(wk_st).astype(bf16),
                "wvs": np.ascontiguousarray(wv_st).astype(bf16),
            }
        )
    return mode, shared, per_core


def _get_nc(mode):
    if mode not in _CACHE:
        _CACHE[mode] = _build(mode)
    return _CACHE[mode]


def kernel(x, wq, wk, wv, wo, freqs_cos, freqs_sin, mask, start_pos=0, **_kw):
    x = np.asarray(x, np.float32)
    wq = np.asarray(wq, np.float32)
    wk = np.asarray(wk, np.float32)
    wv = np.asarray(wv, np.float32)
    wo = np.asarray(wo, np.float32)
    freqs_cos = np.asarray(freqs_cos, np.float32)
    freqs_sin = np.asarray(freqs_sin, np.float32)
    mask = np.asarray(mask, np.float32)

    mode, shared, per_core = _stage(x, wq, wk, wv, wo, freqs_cos, freqs_sin, mask)
    nc = _get_nc(mode)
    in_maps = [dict(shared, **per_core[c]) for c in range(NC)]
    res = run_bass_kernel_spmd(nc, in_maps, core_ids=list(range(NC)))
    outs = [np.asarray(r["out"], np.float32) for r in res.results]
    return np.concatenate(outs, axis=0).reshape(B, S, D)


# revision 24
# speedup vs baseline: 1.0787x; 1.0787x over previous
"""Distributed Trainium2 Bass kernel for llama-style GQA attention block.

B=2, S=2048, D=4096, NH=32, NKV=8, HD=128.  8 NeuronCores, tensor-parallel
over heads (4 q heads + 1 kv head per core), AllToAll to row-sharded layout
before the output projection (avoids the 67MB AllReduce).

All activations live transposed ([feature, seq]) so no on-chip transposes are
needed anywhere except V (computed K-style as V^T then PE-transposed per
128x128 tile): scores are computed as S^T = K-tiles @ Q^T, softmax
normalization is deferred past the PV matmul, and the per-column sums are
reduced with DVE/GpSimd adds plus a single ones-vector matmul on the PE.

Phase 2 is software-pipelined: scores chunks ([128,1024] PSUM, double
buffered) overlap the ScalarE exp of the previous chunk, whose PV matmuls
trail one chunk behind, so the PE never waits on the activation engine.

RoPE is applied in "block" form: the host permutes wq/wk rows within each head
(even pair-components first, odd second) so the on-chip rotate-half is a
partition half-swap (stream_shuffle) instead of a stride-2 interleave.
"""

import sys
import math
import numpy as np

sys.path.insert(0, "/opt/trn_rl_repo")

import ml_dtypes

from concourse import bacc, tile
import concourse.bass as bass
import concourse.mybir as mybir
from concourse.bass_utils import run_bass_kernel_spmd

B, S, D = 2, 2048, 4096
NH, NKV, HD = 32, 8, 128
BS = B * S
NC = 8
NHL = NH // NC          # 4 local q heads
ROWS = BS // NC         # 512 output rows per core
NSC = 8                 # seq chunks of 512 (global rows)
NDT = 32                # D / 128 contraction tiles
KTB = S // HD           # 16 k-tiles per batch
QBB = 4                 # 512-wide q blocks per batch
SCALE = 1.0 / math.sqrt(HD)

F32 = mybir.dt.float32
BF16 = mybir.dt.bfloat16
bf16 = ml_dtypes.bfloat16

# half-swap of the 128 partitions: 32 groups of 4, rotate by 16 groups
SWAP_MASK = [(i + 16) % 32 for i in range(32)]

_CACHE = {}


def _build(mode: str):
    """mode: 'none' (no mask work), 'causal' (skip + binary diag), 'general'."""
    nc = bacc.Bacc("TRN2", target_bir_lowering=False, debug=False, num_devices=NC)

    xT = nc.dram_tensor("xT", [NSC * NDT * 128, 512], BF16, kind="ExternalInput")
    wqs = nc.dram_tensor("wqs", [128, NDT * 512], BF16, kind="ExternalInput")
    wks = nc.dram_tensor("wks", [128, NDT * 128], BF16, kind="ExternalInput")
    wvs = nc.dram_tensor("wvs", [128, NDT * 128], BF16, kind="ExternalInput")
    coss = nc.dram_tensor("coss", [128, S], F32, kind="ExternalInput")
    sins = nc.dram_tensor("sins", [128, S], F32, kind="ExternalInput")
    ident = nc.dram_tensor("ident", [128, 128], BF16, kind="ExternalInput")
    if mode == "causal":
        binsA = nc.dram_tensor("binsA", [128, 4 * 512], BF16, kind="ExternalInput")
    elif mode == "general":
        maskT = nc.dram_tensor("maskT", [S, S], BF16, kind="ExternalInput")
    wos = nc.dram_tensor("wos", [8 * NDT * 128, 512], BF16, kind="ExternalInput")
    out = nc.dram_tensor("out", [ROWS, D], F32, kind="ExternalOutput")

    with tile.TileContext(nc) as tc:
        with (
            tc.tile_pool(name="persist", bufs=1) as persist,
            tc.tile_pool(name="dram", bufs=1, space="DRAM") as dram,
            tc.tile_pool(name="wo_in", bufs=16) as wop,
        ):
            # persistent SBUF tensors
            qt_sb = [persist.tile([128, BS], BF16, tag=f"qt{h}", name=f"qt{h}") for h in range(NHL)]
            kt_sb = persist.tile([128, BS], BF16, tag="kt")
            v_sb = persist.tile([128, BS], BF16, tag="v")
            ones_bf = persist.tile([128, 1], BF16, tag="ones_bf")
            ones_row_bf = persist.tile([1, 128], BF16, tag="ones_row")
            ident_sb = persist.tile([128, 128], BF16, tag="ident")
            nc.vector.memset(ones_bf[:, :], 1.0)
            nc.vector.memset(ones_row_bf[:, :], 1.0)
            nc.sync.dma_start(out=ident_sb[:, :], in_=ident[:, :])

            a2a_in_h = [dram.tile([NC * 128, 512], BF16, name=f"a2ain{h}")
                        for h in range(NHL)]
            a2a_out_h = [dram.tile([NC * 128, 512], BF16, name=f"a2aout{h}")
                         for h in range(NHL)]

            # ---------------- Phase 1: QKV projections + RoPE ----------------
            with (
                tc.tile_pool(name="w1", bufs=1) as w1,
                tc.tile_pool(name="xin", bufs=36) as xin,
                tc.tile_pool(name="ppsum", bufs=3, space="PSUM") as ppsum,
                tc.tile_pool(name="tpsum", bufs=2, space="PSUM") as tpsum,
                tc.tile_pool(name="rope", bufs=4) as rope,
                tc.tile_pool(name="vtsb", bufs=2) as vtsb,
            ):
                wq_sb = w1.tile([128, NDT * 512], BF16, tag="wq")
                wk_sb = w1.tile([128, NDT * 128], BF16, tag="wk")
                wv_sb = w1.tile([128, NDT * 128], BF16, tag="wv")
                cos_sb = w1.tile([128, S], F32, tag="cos")
                sin_sb = w1.tile([128, S], F32, tag="sin")
                # interleave the first seq-chunk's x tiles with the chunked
                # wq loads so the first matmuls start within a few us (a
                # weights-first order queues 8.5MB of DMA ahead of x)
                first_x = {}
                for c in range(8):
                    nc.sync.dma_start(
                        out=wq_sb[:, c * 2048:(c + 1) * 2048],
                        in_=wqs[:, c * 2048:(c + 1) * 2048],
                    )
                    for dt in range(4 * c, 4 * c + 4):
                        xt = xin.tile([128, 512], BF16, tag="xt", name=f"xt{dt}")
                        nc.sync.dma_start(out=xt[:, :], in_=xT[dt * 128:(dt + 1) * 128, :])
                        first_x[dt] = xt
                nc.sync.dma_start(out=cos_sb[:, :], in_=coss[:, :])
                nc.sync.dma_start(out=sin_sb[:, :], in_=sins[:, :])
                nc.sync.dma_start(out=wk_sb[:, :], in_=wks[:, :])
                nc.sync.dma_start(out=wv_sb[:, :], in_=wvs[:, :])

                for sc in range(NSC):
                    xts = []
                    for dt in range(NDT):
                        if sc == 0:
                            xts.append(first_x[dt])
                            continue
                        xt = xin.tile([128, 512], BF16, tag="xt", name=f"xt{dt}")
                        g = sc * NDT + dt
                        nc.sync.dma_start(
                            out=xt[:, :], in_=xT[g * 128:(g + 1) * 128, :]
                        )
                        xts.append(xt)
                    pos = (sc % 4) * 512

                    def rope_drain(src, dst):
                        rot = rope.tile([128, 512], F32, tag="rot", name="rot")
                        t1 = rope.tile([128, 512], F32, tag="t1", name="t1")
                        nc.vector.stream_shuffle(
                            out=rot[:, :], in_=src[:, :], mask=SWAP_MASK
                        )
                        nc.vector.tensor_mul(
                            out=t1[:, :], in0=src[:, :],
                            in1=cos_sb[:, pos:pos + 512],
                        )
                        nc.vector.tensor_mul(
                            out=rot[:, :], in0=rot[:, :],
                            in1=sin_sb[:, pos:pos + 512],
                        )
                        nc.vector.tensor_add(
                            out=dst[:, sc * 512:(sc + 1) * 512],
                            in0=t1[:, :], in1=rot[:, :],
                        )

                    # each accumulation group gets its own psum tile, groups
                    # run back-to-back (interleaved groups on one tile break
                    # PSUM has_written semantics)
                    for h in range(NHL):
                        pq = ppsum.tile([128, 512], F32, tag="pp", name="pq")
                        for dt in range(NDT):
                            nc.tensor.matmul(
                                out=pq[:, :],
                                lhsT=wq_sb[:, dt * 512 + h * 128:dt * 512 + (h + 1) * 128],
                                rhs=xts[dt][:, :], start=dt == 0, stop=dt == NDT - 1,
                            )
                        rope_drain(pq, qt_sb[h])
                    pk = ppsum.tile([128, 512], F32, tag="pp", name="pk")
                    for dt in range(NDT):
                        nc.tensor.matmul(
                            out=pk[:, :],
                            lhsT=wk_sb[:, dt * 128:(dt + 1) * 128],
                            rhs=xts[dt][:, :], start=dt == 0, stop=dt == NDT - 1,
                        )
                    rope_drain(pk, kt_sb)
                    # V^T K-style (N=512 streams), then PE-transpose per tile
                    pv = ppsum.tile([128, 512], F32, tag="pp", name="pv")
                    for dt in range(NDT):
                        nc.tensor.matmul(
                            out=pv[:, :],
                            lhsT=wv_sb[:, dt * 128:(dt + 1) * 128],
                            rhs=xts[dt][:, :], start=dt == 0, stop=dt == NDT - 1,
                        )
                    vt = vtsb.tile([128, 512], BF16, tag="vt")
                    nc.vector.tensor_copy(out=vt[:, :], in_=pv[:, :])
                    for st in range(4):
                        tp = tpsum.tile([128, 128], BF16, tag="tp", name="tp")
                        nc.tensor.transpose(
                            tp[:, :], vt[:, st * 128:(st + 1) * 128], ident_sb[:, :]
                        )
                        nc.vector.tensor_copy(
                            out=v_sb[:, (sc * 4 + st) * 128:(sc * 4 + st + 1) * 128],
                            in_=tp[:, :],
                        )

            # ---------------- Phase 2: attention ----------------
            with (
                tc.tile_pool(name="p2sb", bufs=1) as p2sb,
                tc.tile_pool(name="probs", bufs=2) as probsp,
                tc.tile_pool(name="sacc", bufs=2) as saccp,
                tc.tile_pool(name="spsum", bufs=2, space="PSUM") as spsum,
                tc.tile_pool(name="otpsum", bufs=2, space="PSUM") as otpsum,
                tc.tile_pool(name="smpsum", bufs=1, space="PSUM") as smpsum,
                tc.tile_pool(name="bcpsum", bufs=1, space="PSUM") as bcpsum,
                tc.tile_pool(name="aosb", bufs=4) as aosb,
                tc.tile_pool(name="msksb", bufs=4) as msksb,
            ):
                if mode == "causal":
                    bin_sb = p2sb.tile([128, 4 * 512], BF16, tag="bin")
                    nc.sync.dma_start(out=bin_sb[:, :], in_=binsA[:, :])
                for h in range(NHL):
                    pending = []
                    for b in range(B):
                        for qb in range(QBB):
                            nkt = 4 * qb + 4 if mode == "causal" else KTB
                            nch = nkt // 2
                            qs = qt_sb[h][:, b * S + qb * 512:b * S + (qb + 1) * 512]
                            probs = probsp.tile([128, KTB * 512], BF16, tag="probs")
                            ot = otpsum.tile([128, 512], F32, tag="ot")
                            st = saccp.tile([128, 8 * 512], BF16, tag="st", name="st")

                            def scores(ch):
                                # 2 matmuls into the 2 bank-aligned halves of
                                # one [128, 1024] psum, one exp.  On diagonal
                                # tiles the causal mask is accumulated on the
                                # PE (ident^T @ (-1e9 * upper)) so no vector
                                # engine touches the probs path.
                                ps = spsum.tile([128, 1024], F32, tag="ps", name="ps")
                                for m in range(2):
                                    kt = 2 * ch + m
                                    diag = mode == "causal" and ch >= nch - 2
                                    nc.tensor.matmul(
                                        out=ps[:, m * 512:(m + 1) * 512],
                                        lhsT=kt_sb[:, (b * KTB + kt) * 128:(b * KTB + kt + 1) * 128],
                                        rhs=qs, start=True, stop=not diag,
                                    )
                                    if diag:
                                        dm = (ch - (nch - 2)) * 2 + m
                                        nc.tensor.matmul(
                                            out=ps[:, m * 512:(m + 1) * 512],
                                            lhsT=ident_sb[:, :],
                                            rhs=bin_sb[:, dm * 512:(dm + 1) * 512],
                                            start=False, stop=True,
                                        )
                                    if mode == "general":
                                        mt = msksb.tile([128, 512], BF16, tag="mt")
                                        nc.sync.dma_start(
                                            out=mt[:, :],
                                            in_=maskT[kt * 128:(kt + 1) * 128,
                                                      qb * 512:(qb + 1) * 512],
                                        )
                                        nc.vector.tensor_add(
                                            out=ps[:, m * 512:(m + 1) * 512],
                                            in0=ps[:, m * 512:(m + 1) * 512],
                                            in1=mt[:, :],
                                        )
                                pslice = probs[:, ch * 1024:(ch + 1) * 1024]
                                nc.scalar.activation(
                                    pslice, ps[:, :],
                                    mybir.ActivationFunctionType.Exp,
                                    bias=0.0, scale=SCALE,
                                )

                            def post(ch):
                                # PV for the 2 tiles of chunk ch
                                for m in range(2):
                                    kt = 2 * ch + m
                                    nc.tensor.matmul(
                                        out=ot[:, :],
                                        lhsT=v_sb[:, (b * KTB + kt) * 128:(b * KTB + kt + 1) * 128],
                                        rhs=probs[:, kt * 512:(kt + 1) * 512],
                                        start=kt == 0, stop=kt == nkt - 1,
                                    )

                            def tree(nkt=nkt, qb=qb, probs=probs, st=st):
                                # pairwise-halving softmax-sum tree: wide bf16
                                # adds; small blocks go to GpSimd, big ones to
                                # the (faster) DVE, balancing the two engines
                                eng = nc.gpsimd if qb < 2 else nc.vector
                                cols = nkt * 512
                                if nkt == 12:
                                    eng.tensor_add(
                                        out=st[:, 0:2048], in0=probs[:, 0:2048],
                                        in1=probs[:, 2048:4096])
                                    eng.tensor_add(
                                        out=st[:, 0:2048], in0=st[:, 0:2048],
                                        in1=probs[:, 4096:6144])
                                    w = 2048
                                else:
                                    w = cols // 2
                                    eng.tensor_add(
                                        out=st[:, 0:w], in0=probs[:, 0:w],
                                        in1=probs[:, w:cols])
                                while w > 512:
                                    hw = w // 2
                                    eng.tensor_add(
                                        out=st[:, 0:hw], in0=st[:, 0:hw],
                                        in1=st[:, hw:w])
                                    w = hw

                            state = {}

                            def tail1(st=st, state=state):
                                # denominators: cross-partition ones-reduction
                                # of the tree sum on the PE + fast reciprocal
                                sm = smpsum.tile([1, 512], F32, tag="sm")
                                nc.tensor.matmul(
                                    out=sm[:, :], lhsT=ones_bf[:, :],
                                    rhs=st[:, 0:512],
                                    start=True, stop=True,
                                )
                                rec32 = aosb.tile([1, 512], F32, tag="rec32")
                                nc.vector.reciprocal_approx_fast(
                                    out=rec32[:, :], in_=sm[:, :]
                                )
                                rec = aosb.tile([1, 512], BF16, tag="rec")
                                nc.scalar.copy(out=rec[:, :], in_=rec32[:, :])
                                state["rec"] = rec

                            def tail2(h=h, b=b, qb=qb, ot=ot, state=state):
                                # broadcast 1/denom to 128 partitions and
                                # normalize the PV accumulator
                                bc = bcpsum.tile([128, 512], F32, tag="bc")
                                nc.tensor.matmul(
                                    out=bc[:, :], lhsT=ones_row_bf[:, :],
                                    rhs=state["rec"][:, :],
                                    start=True, stop=True,
                                )
                                bc_sb = aosb.tile([128, 512], F32, tag="bc_sb")
                                nc.vector.tensor_copy(out=bc_sb[:, :], in_=bc[:, :])
                                ao = aosb.tile([128, 512], BF16, tag="ao")
                                nc.vector.tensor_mul(
                                    out=ao[:, :], in0=ot[:, :], in1=bc_sb[:, :]
                                )
                                j = b * 4 + qb
                                nc.sync.dma_start(
                                    out=a2a_in_h[h][j * 128:(j + 1) * 128, :],
                                    in_=ao[:, :],
                                )

                            # the previous block's tail stages fire after this
                            # block's scores chunks so their sm/bc matmuls
                            # never stall the PE behind the DVE reductions
                            for ch in range(nch):
                                scores(ch)
                                if pending:
                                    pending.pop(0)()
                                if ch > 0:
                                    post(ch - 1)
                            post(nch - 1)
                            tree()
                            pending += [tail1, tail2]
                    for fn in pending:
                        fn()
                    pending = []
                    # per-head AllToAll: overlaps with the next head's compute
                    nc.gpsimd.collective_compute(
                        "AllToAll", mybir.AluOpType.bypass,
                        ins=[a2a_in_h[h].opt()], outs=[a2a_out_h[h].opt()],
                        replica_groups=[list(range(NC))],
                    )

            # ---------------- Phase 3: output projection ----------
            with (
                tc.tile_pool(name="attsb", bufs=1) as attp,
                tc.tile_pool(name="ypsum", bufs=8, space="PSUM") as ypsum,
                tc.tile_pool(name="ysb", bufs=4) as ysbp,
            ):
                att_sb = attp.tile([128, NDT * 512], BF16, tag="att")
                # h-major order: tiles for head h usable right after A2A #h
                t_order = [i * 4 + h for h in range(NHL) for i in range(NC)]
                for t in t_order:
                    i, h = t // 4, t % 4
                    nc.sync.dma_start(
                        out=att_sb[:, t * 512:(t + 1) * 512],
                        in_=a2a_out_h[h][i * 128:(i + 1) * 128, :],
                    )

                yps_dc = {}

                def wo_mms(dc, ts, n0):
                    yps = yps_dc[dc]
                    for n, t in enumerate(ts):
                        wot = wop.tile([128, 512], BF16, tag="wot")
                        g = dc * NDT + t
                        nc.sync.dma_start(
                            out=wot[:, :], in_=wos[g * 128:(g + 1) * 128, :]
                        )
                        for st in range(4):
                            nc.tensor.matmul(
                                out=yps[st][:, :],
                                lhsT=att_sb[:, t * 512 + st * 128:t * 512 + (st + 1) * 128],
                                rhs=wot[:, :],
                                start=n0 + n == 0, stop=n0 + n == NDT - 1,
                            )

                def drain(dc):
                    for st in range(4):
                        ysb = ysbp.tile([128, 512], F32, tag="ysb")
                        nc.vector.tensor_copy(out=ysb[:, :], in_=yps_dc[dc][st][:, :])
                        nc.sync.dma_start(
                            out=out[st * 128:(st + 1) * 128, dc * 512:(dc + 1) * 512],
                            in_=ysb[:, :],
                        )

                # heads 0-2 of dc before head 3 of dc-1: the final A2A hides
                # behind two dc passes worth of head-0-2 accumulation
                for dc in range(8):
                    yps_dc[dc] = [
                        ypsum.tile([128, 512], F32, tag="yp", name=f"yp{_s}")
                        for _s in range(4)
                    ]
                    wo_mms(dc, t_order[:24], 0)
                    if dc >= 1:
                        wo_mms(dc - 1, t_order[24:], 24)
                        drain(dc - 1)
                wo_mms(7, t_order[24:], 24)
                drain(7)
    nc.compile()
    return nc


# within each head: 4 windows of 32 partitions = [16 re-pairs | 16 im-pairs],
# so the rotate-half is stream_shuffle's per-32-window rotation by 16.
_PERM_IDX = np.array(
    [
        2 * (w * 16 + (j if j < 16 else j - 16)) + (0 if j < 16 else 1)
        for w in range(4)
        for j in range(32)
    ]
)
_PI = np.array([w * 16 + (j if j < 16 else j - 16) for w in range(4) for j in range(32)])
_SGN = np.array(
    [(-1.0 if j < 16 else 1.0) for w in range(4) for j in range(32)], np.float32
)


def _perm_block(w):
    o = w.reshape(-1, HD, D)
    return o[:, _PERM_IDX, :].reshape(-1, D)


def _stage(x, wq, wk, wv, wo, freqs_cos, freqs_sin, mask):
    """Returns (mode, shared dict, per-core dicts)."""
    causal = np.where(np.triu(np.ones((S, S), dtype=bool), k=1), -1e9, 0.0).astype(
        np.float32
    )
    if not mask.any():
        mode = "none"
    elif np.array_equal(mask, causal):
        mode = "causal"
    else:
        mode = "general"

    xT = np.ascontiguousarray(x.reshape(BS, D).T)  # [D, BS]
    x_st = (
        xT.reshape(NDT, 128, NSC, 512).transpose(2, 0, 1, 3).reshape(NSC * NDT * 128, 512)
    ).astype(bf16)
    woT = np.ascontiguousarray(wo.T)  # [hd, Dout]
    wo_st = (
        woT.reshape(NDT, 128, 8, 512).transpose(2, 0, 1, 3).reshape(8 * NDT * 128, 512)
    ).astype(bf16)
    cosT = freqs_cos.T.astype(np.float32)  # [64, S]
    sinT = freqs_sin.T.astype(np.float32)
    cos_st = np.ascontiguousarray(cosT[_PI, :])
    sin_st = np.ascontiguousarray(sinT[_PI, :] * _SGN[:, None])

    shared = {"xT": x_st, "coss": cos_st, "sins": sin_st, "wos": wo_st,
              "ident": np.eye(128, dtype=bf16)}
    if mode == "causal":
        # additive mask for the diagonal 512x512 block in [k, q] layout:
        # -1e9 where k > q (future), accumulated into the scores psum
        add_diag = np.where(
            np.tril(np.ones((512, 512), dtype=bool), -1), -1e9, 0.0
        ).astype(np.float32).astype(bf16)
        shared["binsA"] = np.ascontiguousarray(
            add_diag.reshape(4, 128, 512).transpose(1, 0, 2).reshape(128, 2048)
        )
    elif mode == "general":
        shared["maskT"] = np.ascontiguousarray(mask.T * math.sqrt(HD)).astype(bf16)

    per_core = []
    for c in range(NC):
        wq_c = _perm_block(wq[c * 512:(c + 1) * 512]).T  # [D, 512]
        wk_c = _perm_block(wk[c * 128:(c + 1) * 128]).T  # [D, 128]
        wv_c = wv[c * 128:(c + 1) * 128].T               # [D, 128]
        wq_st = wq_c.reshape(NDT, 128, 512).transpose(1, 0, 2).reshape(128, NDT * 512)
        wk_st = wk_c.reshape(NDT, 128, 128).transpose(1, 0, 2).reshape(128, NDT * 128)
        wv_st = wv_c.reshape(NDT, 128, 128).transpose(1, 0, 2).reshape(128, NDT * 128)
        per_core.append(
            {
                "wqs": np.ascontiguousarray(wq_st).astype(bf16),
                "wks": np.ascontiguousarray(wk_st).astype(bf16),
                "wvs": np.ascontiguousarray(wv_st).astype(bf16),
            }
        )
    return mode, shared, per_core


def _get_nc(mode):
    if mode not in _CACHE:
        _CACHE[mode] = _build(mode)
    return _CACHE[mode]


def kernel(x, wq, wk, wv, wo, freqs_cos, freqs_sin, mask, start_pos=0, **_kw):
    x = np.asarray(x, np.float32)
    wq = np.asarray(wq, np.float32)
    wk = np.asarray(wk, np.float32)
    wv = np.asarray(wv, np.float32)
    wo = np.asarray(wo, np.float32)
    freqs_cos = np.asarray(freqs_cos, np.float32)
    freqs_sin = np.asarray(freqs_sin, np.float32)
    mask = np.asarray(mask, np.float32)

    mode, shared, per_core = _stage(x, wq, wk, wv, wo, freqs_cos, freqs_sin, mask)
    nc = _get_nc(mode)
    in_maps = [dict(shared, **per_core[c]) for c in range(NC)]
    res = run_bass_kernel_spmd(nc, in_maps, core_ids=list(range(NC)))
    outs = [np.asarray(r["out"], np.float32) for r in res.results]
    return np.concatenate(outs, axis=0).reshape(B, S, D)


# revision 26
# speedup vs baseline: 1.1961x; 1.1088x over previous
"""Distributed Trainium2 Bass kernel for llama-style GQA attention block.

B=2, S=2048, D=4096, NH=32, NKV=8, HD=128.  8 NeuronCores, tensor-parallel
over heads (4 q heads + 1 kv head per core), AllToAll to row-sharded layout
before the output projection (avoids the 67MB AllReduce).

All activations live transposed ([feature, seq]) so no on-chip transposes are
needed anywhere except V (computed K-style as V^T then PE-transposed per
128x128 tile): scores are computed as S^T = K-tiles @ Q^T, softmax
normalization is deferred past the PV matmul, and the per-column sums are
reduced with DVE/GpSimd adds plus a single ones-vector matmul on the PE.

Phase 2 is software-pipelined: scores chunks ([128,1024] PSUM, double
buffered) overlap the ScalarE exp of the previous chunk, whose PV matmuls
trail one chunk behind, so the PE never waits on the activation engine.

RoPE is applied in "block" form: the host permutes wq/wk rows within each head
(even pair-components first, odd second) so the on-chip rotate-half is a
partition half-swap (stream_shuffle) instead of a stride-2 interleave.
"""

import sys
import math
import numpy as np

sys.path.insert(0, "/opt/trn_rl_repo")

import ml_dtypes

from concourse import bacc, tile
import concourse.bass as bass
import concourse.mybir as mybir
from concourse.bass_utils import run_bass_kernel_spmd

B, S, D = 2, 2048, 4096
NH, NKV, HD = 32, 8, 128
BS = B * S
NC = 8
NHL = NH // NC          # 4 local q heads
ROWS = BS // NC         # 512 output rows per core
NSC = 8                 # seq chunks of 512 (global rows)
NDT = 32                # D / 128 contraction tiles
KTB = S // HD           # 16 k-tiles per batch
QBB = 4                 # 512-wide q blocks per batch
SCALE = 1.0 / math.sqrt(HD)

F32 = mybir.dt.float32
BF16 = mybir.dt.bfloat16
bf16 = ml_dtypes.bfloat16

# half-swap of the 128 partitions: 32 groups of 4, rotate by 16 groups
SWAP_MASK = [(i + 16) % 32 for i in range(32)]

_CACHE = {}


def _build(mode: str):
    """mode: 'none' (no mask work), 'causal' (skip + binary diag), 'general'."""
    nc = bacc.Bacc("TRN2", target_bir_lowering=False, debug=False, num_devices=NC)

    xT = nc.dram_tensor("xT", [NSC * NDT * 128, 512], BF16, kind="ExternalInput")
    wqs = nc.dram_tensor("wqs", [128, NDT * 512], BF16, kind="ExternalInput")
    wks = nc.dram_tensor("wks", [128, NDT * 128], BF16, kind="ExternalInput")
    wvs = nc.dram_tensor("wvs", [128, NDT * 128], BF16, kind="ExternalInput")
    coss = nc.dram_tensor("coss", [128, S], F32, kind="ExternalInput")
    sins = nc.dram_tensor("sins", [128, S], F32, kind="ExternalInput")
    ident = nc.dram_tensor("ident", [128, 128], BF16, kind="ExternalInput")
    if mode == "causal":
        binsA = nc.dram_tensor("binsA", [128, 4 * 512], BF16, kind="ExternalInput")
    elif mode == "general":
        maskT = nc.dram_tensor("maskT", [S, S], BF16, kind="ExternalInput")
    wos = nc.dram_tensor("wos", [8 * NDT * 128, 512], BF16, kind="ExternalInput")
    out = nc.dram_tensor("out", [ROWS, D], F32, kind="ExternalOutput")

    with tile.TileContext(nc) as tc:
        with (
            tc.tile_pool(name="persist", bufs=1) as persist,
            tc.tile_pool(name="dram", bufs=1, space="DRAM") as dram,
            tc.tile_pool(name="wo_in", bufs=16) as wop,
        ):
            # persistent SBUF tensors
            qt_sb = [persist.tile([128, BS], BF16, tag=f"qt{h}", name=f"qt{h}") for h in range(NHL)]
            kt_sb = persist.tile([128, BS], BF16, tag="kt")
            v_sb = persist.tile([128, BS], BF16, tag="v")
            ones_bf = persist.tile([128, 1], BF16, tag="ones_bf")
            ones_row_bf = persist.tile([1, 128], BF16, tag="ones_row")
            ident_sb = persist.tile([128, 128], BF16, tag="ident")
            nc.vector.memset(ones_bf[:, :], 1.0)
            nc.vector.memset(ones_row_bf[:, :], 1.0)
            nc.sync.dma_start(out=ident_sb[:, :], in_=ident[:, :])

            a2a_in_h = [dram.tile([NC * 128, 512], BF16, name=f"a2ain{h}")
                        for h in range(NHL)]
            a2a_out_h = [dram.tile([NC * 128, 512], BF16, name=f"a2aout{h}")
                         for h in range(NHL)]

            # ---------------- Phase 1: QKV projections + RoPE ----------------
            with (
                tc.tile_pool(name="w1", bufs=1) as w1,
                tc.tile_pool(name="xin", bufs=36) as xin,
                tc.tile_pool(name="ppsum", bufs=3, space="PSUM") as ppsum,
                tc.tile_pool(name="tpsum", bufs=2, space="PSUM") as tpsum,
                tc.tile_pool(name="rope", bufs=4) as rope,
                tc.tile_pool(name="vtsb", bufs=2) as vtsb,
            ):
                wq_sb = w1.tile([128, NDT * 512], BF16, tag="wq")
                wk_sb = w1.tile([128, NDT * 128], BF16, tag="wk")
                wv_sb = w1.tile([128, NDT * 128], BF16, tag="wv")
                cos_sb = w1.tile([128, S], F32, tag="cos")
                sin_sb = w1.tile([128, S], F32, tag="sin")
                # interleave the first seq-chunk's x tiles with the chunked
                # wq loads so the first matmuls start within a few us (a
                # weights-first order queues 8.5MB of DMA ahead of x)
                first_x = {}
                for c in range(8):
                    nc.sync.dma_start(
                        out=wq_sb[:, c * 2048:(c + 1) * 2048],
                        in_=wqs[:, c * 2048:(c + 1) * 2048],
                    )
                    for dt in range(4 * c, 4 * c + 4):
                        xt = xin.tile([128, 512], BF16, tag="xt", name=f"xt{dt}")
                        nc.sync.dma_start(out=xt[:, :], in_=xT[dt * 128:(dt + 1) * 128, :])
                        first_x[dt] = xt
                nc.sync.dma_start(out=cos_sb[:, :], in_=coss[:, :])
                nc.sync.dma_start(out=sin_sb[:, :], in_=sins[:, :])
                nc.sync.dma_start(out=wk_sb[:, :], in_=wks[:, :])
                nc.sync.dma_start(out=wv_sb[:, :], in_=wvs[:, :])

                for sc in range(NSC):
                    xts = []
                    for dt in range(NDT):
                        if sc == 0:
                            xts.append(first_x[dt])
                            continue
                        xt = xin.tile([128, 512], BF16, tag="xt", name=f"xt{dt}")
                        g = sc * NDT + dt
                        nc.sync.dma_start(
                            out=xt[:, :], in_=xT[g * 128:(g + 1) * 128, :]
                        )
                        xts.append(xt)
                    pos = (sc % 4) * 512

                    def rope_drain(src, dst):
                        rot = rope.tile([128, 512], F32, tag="rot", name="rot")
                        t1 = rope.tile([128, 512], F32, tag="t1", name="t1")
                        nc.vector.stream_shuffle(
                            out=rot[:, :], in_=src[:, :], mask=SWAP_MASK
                        )
                        nc.vector.tensor_mul(
                            out=t1[:, :], in0=src[:, :],
                            in1=cos_sb[:, pos:pos + 512],
                        )
                        nc.vector.tensor_mul(
                            out=rot[:, :], in0=rot[:, :],
                            in1=sin_sb[:, pos:pos + 512],
                        )
                        nc.vector.tensor_add(
                            out=dst[:, sc * 512:(sc + 1) * 512],
                            in0=t1[:, :], in1=rot[:, :],
                        )

                    # each accumulation group gets its own psum tile, groups
                    # run back-to-back (interleaved groups on one tile break
                    # PSUM has_written semantics)
                    for h in range(NHL):
                        pq = ppsum.tile([128, 512], F32, tag="pp", name="pq")
                        for dt in range(NDT):
                            nc.tensor.matmul(
                                out=pq[:, :],
                                lhsT=wq_sb[:, dt * 512 + h * 128:dt * 512 + (h + 1) * 128],
                                rhs=xts[dt][:, :], start=dt == 0, stop=dt == NDT - 1,
                            )
                        rope_drain(pq, qt_sb[h])
                    pk = ppsum.tile([128, 512], F32, tag="pp", name="pk")
                    for dt in range(NDT):
                        nc.tensor.matmul(
                            out=pk[:, :],
                            lhsT=wk_sb[:, dt * 128:(dt + 1) * 128],
                            rhs=xts[dt][:, :], start=dt == 0, stop=dt == NDT - 1,
                        )
                    rope_drain(pk, kt_sb)
                    # V^T K-style (N=512 streams), then PE-transpose per tile
                    pv = ppsum.tile([128, 512], F32, tag="pp", name="pv")
                    for dt in range(NDT):
                        nc.tensor.matmul(
                            out=pv[:, :],
                            lhsT=wv_sb[:, dt * 128:(dt + 1) * 128],
                            rhs=xts[dt][:, :], start=dt == 0, stop=dt == NDT - 1,
                        )
                    vt = vtsb.tile([128, 512], BF16, tag="vt")
                    nc.vector.tensor_copy(out=vt[:, :], in_=pv[:, :])
                    for st in range(4):
                        tp = tpsum.tile([128, 128], BF16, tag="tp", name="tp")
                        nc.tensor.transpose(
                            tp[:, :], vt[:, st * 128:(st + 1) * 128], ident_sb[:, :]
                        )
                        nc.vector.tensor_copy(
                            out=v_sb[:, (sc * 4 + st) * 128:(sc * 4 + st + 1) * 128],
                            in_=tp[:, :],
                        )

            # ---------------- Phase 2: attention ----------------
            with (
                tc.tile_pool(name="p2sb", bufs=1) as p2sb,
                tc.tile_pool(name="probs", bufs=2) as probsp,
                tc.tile_pool(name="sacc", bufs=2) as saccp,
                tc.tile_pool(name="spsum", bufs=2, space="PSUM") as spsum,
                tc.tile_pool(name="otpsum", bufs=2, space="PSUM") as otpsum,
                tc.tile_pool(name="smpsum", bufs=1, space="PSUM") as smpsum,
                tc.tile_pool(name="bcpsum", bufs=1, space="PSUM") as bcpsum,
                tc.tile_pool(name="aosb", bufs=4) as aosb,
                tc.tile_pool(name="msksb", bufs=4) as msksb,
            ):
                if mode == "causal":
                    bin_sb = p2sb.tile([128, 4 * 512], BF16, tag="bin")
                    nc.sync.dma_start(out=bin_sb[:, :], in_=binsA[:, :])
                for h in range(NHL):
                    pending = []
                    for b in range(B):
                        for qb in range(QBB):
                            nkt = 4 * qb + 4 if mode == "causal" else KTB
                            nch = nkt // 2
                            qs = qt_sb[h][:, b * S + qb * 512:b * S + (qb + 1) * 512]
                            probs = probsp.tile([128, KTB * 512], BF16, tag="probs")
                            ot = otpsum.tile([128, 512], F32, tag="ot")
                            st = saccp.tile([128, 8 * 512], BF16, tag="st", name="st")

                            def scores(ch):
                                # 2 matmuls into the 2 bank-aligned halves of
                                # one [128, 1024] psum, one exp.  On diagonal
                                # tiles the causal mask is accumulated on the
                                # PE (ident^T @ (-1e9 * upper)) so no vector
                                # engine touches the probs path.
                                ps = spsum.tile([128, 1024], F32, tag="ps", name="ps")
                                for m in range(2):
                                    kt = 2 * ch + m
                                    diag = mode == "causal" and ch >= nch - 2
                                    nc.tensor.matmul(
                                        out=ps[:, m * 512:(m + 1) * 512],
                                        lhsT=kt_sb[:, (b * KTB + kt) * 128:(b * KTB + kt + 1) * 128],
                                        rhs=qs, start=True, stop=not diag,
                                    )
                                    if diag:
                                        dm = (ch - (nch - 2)) * 2 + m
                                        nc.tensor.matmul(
                                            out=ps[:, m * 512:(m + 1) * 512],
                                            lhsT=ident_sb[:, :],
                                            rhs=bin_sb[:, dm * 512:(dm + 1) * 512],
                                            start=False, stop=True,
                                        )
                                    if mode == "general":
                                        mt = msksb.tile([128, 512], BF16, tag="mt")
                                        nc.sync.dma_start(
                                            out=mt[:, :],
                                            in_=maskT[kt * 128:(kt + 1) * 128,
                                                      qb * 512:(qb + 1) * 512],
                                        )
                                        nc.vector.tensor_add(
                                            out=ps[:, m * 512:(m + 1) * 512],
                                            in0=ps[:, m * 512:(m + 1) * 512],
                                            in1=mt[:, :],
                                        )
                                pslice = probs[:, ch * 1024:(ch + 1) * 1024]
                                nc.scalar.activation(
                                    pslice, ps[:, :],
                                    mybir.ActivationFunctionType.Exp,
                                    bias=0.0, scale=SCALE,
                                )

                            def post(ch):
                                # PV for the 2 tiles of chunk ch
                                for m in range(2):
                                    kt = 2 * ch + m
                                    nc.tensor.matmul(
                                        out=ot[:, :],
                                        lhsT=v_sb[:, (b * KTB + kt) * 128:(b * KTB + kt + 1) * 128],
                                        rhs=probs[:, kt * 512:(kt + 1) * 512],
                                        start=kt == 0, stop=kt == nkt - 1,
                                    )

                            def tree(nkt=nkt, qb=qb, probs=probs, st=st):
                                # pairwise-halving softmax-sum tree: wide bf16
                                # adds; small blocks go to GpSimd, big ones to
                                # the (faster) DVE, balancing the two engines
                                eng = nc.vector
                                cols = nkt * 512
                                if nkt == 12:
                                    eng.tensor_add(
                                        out=st[:, 0:2048], in0=probs[:, 0:2048],
                                        in1=probs[:, 2048:4096])
                                    eng.tensor_add(
                                        out=st[:, 0:2048], in0=st[:, 0:2048],
                                        in1=probs[:, 4096:6144])
                                    w = 2048
                                else:
                                    w = cols // 2
                                    eng.tensor_add(
                                        out=st[:, 0:w], in0=probs[:, 0:w],
                                        in1=probs[:, w:cols])
                                while w > 512:
                                    hw = w // 2
                                    eng.tensor_add(
                                        out=st[:, 0:hw], in0=st[:, 0:hw],
                                        in1=st[:, hw:w])
                                    w = hw

                            state = {}

                            def tail1(st=st, state=state):
                                # denominators: cross-partition ones-reduction
                                # of the tree sum on the PE + fast reciprocal
                                sm = smpsum.tile([1, 512], F32, tag="sm")
                                nc.tensor.matmul(
                                    out=sm[:, :], lhsT=ones_bf[:, :],
                                    rhs=st[:, 0:512],
                                    start=True, stop=True,
                                )
                                rec32 = aosb.tile([1, 512], F32, tag="rec32")
                                nc.vector.reciprocal_approx_fast(
                                    out=rec32[:, :], in_=sm[:, :]
                                )
                                rec = aosb.tile([1, 512], BF16, tag="rec")
                                nc.scalar.copy(out=rec[:, :], in_=rec32[:, :])
                                state["rec"] = rec

                            def tail2(h=h, b=b, qb=qb, ot=ot, state=state):
                                # broadcast 1/denom to 128 partitions and
                                # normalize the PV accumulator
                                bc = bcpsum.tile([128, 512], F32, tag="bc")
                                nc.tensor.matmul(
                                    out=bc[:, :], lhsT=ones_row_bf[:, :],
                                    rhs=state["rec"][:, :],
                                    start=True, stop=True,
                                )
                                bc_sb = aosb.tile([128, 512], F32, tag="bc_sb")
                                nc.vector.tensor_copy(out=bc_sb[:, :], in_=bc[:, :])
                                ao = aosb.tile([128, 512], BF16, tag="ao")
                                nc.vector.tensor_mul(
                                    out=ao[:, :], in0=ot[:, :], in1=bc_sb[:, :]
                                )
                                j = b * 4 + qb
                                nc.sync.dma_start(
                                    out=a2a_in_h[h][j * 128:(j + 1) * 128, :],
                                    in_=ao[:, :],
                                )

                            # the previous block's tail stages fire at the
                            # LAST two chunks of this block, giving its DVE
                            # sum-tree maximum slack before the sm/bc matmuls
                            # hit the PE queue
                            for ch in range(nch):
                                scores(ch)
                                if pending and ch >= nch - 2:
                                    pending.pop(0)()
                                if ch > 0:
                                    post(ch - 1)
                            post(nch - 1)
                            tree()
                            pending += [tail1, tail2]
                    for fn in pending:
                        fn()
                    pending = []
                    # per-head AllToAll: overlaps with the next head's compute
                    nc.gpsimd.collective_compute(
                        "AllToAll", mybir.AluOpType.bypass,
                        ins=[a2a_in_h[h].opt()], outs=[a2a_out_h[h].opt()],
                        replica_groups=[list(range(NC))],
                    )

            # ---------------- Phase 3: output projection ----------
            with (
                tc.tile_pool(name="attsb", bufs=1) as attp,
                tc.tile_pool(name="ypsum", bufs=8, space="PSUM") as ypsum,
                tc.tile_pool(name="ysb", bufs=4) as ysbp,
            ):
                att_sb = attp.tile([128, NDT * 512], BF16, tag="att")
                # h-major order: tiles for head h usable right after A2A #h
                t_order = [i * 4 + h for h in range(NHL) for i in range(NC)]
                for t in t_order:
                    i, h = t // 4, t % 4
                    nc.sync.dma_start(
                        out=att_sb[:, t * 512:(t + 1) * 512],
                        in_=a2a_out_h[h][i * 128:(i + 1) * 128, :],
                    )

                yps_dc = {}

                def wo_mms(dc, ts, n0):
                    yps = yps_dc[dc]
                    for n, t in enumerate(ts):
                        wot = wop.tile([128, 512], BF16, tag="wot")
                        g = dc * NDT + t
                        nc.sync.dma_start(
                            out=wot[:, :], in_=wos[g * 128:(g + 1) * 128, :]
                        )
                        for st in range(4):
                            nc.tensor.matmul(
                                out=yps[st][:, :],
                                lhsT=att_sb[:, t * 512 + st * 128:t * 512 + (st + 1) * 128],
                                rhs=wot[:, :],
                                start=n0 + n == 0, stop=n0 + n == NDT - 1,
                            )

                def drain(dc):
                    for st in range(4):
                        ysb = ysbp.tile([128, 512], F32, tag="ysb")
                        nc.vector.tensor_copy(out=ysb[:, :], in_=yps_dc[dc][st][:, :])
                        nc.sync.dma_start(
                            out=out[st * 128:(st + 1) * 128, dc * 512:(dc + 1) * 512],
                            in_=ysb[:, :],
                        )

                # heads 0-2 of dc before head 3 of dc-1: the final A2A hides
                # behind two dc passes worth of head-0-2 accumulation
                for dc in range(8):
                    yps_dc[dc] = [
                        ypsum.tile([128, 512], F32, tag="yp", name=f"yp{_s}")
                        for _s in range(4)
                    ]
                    wo_mms(dc, t_order[:24], 0)
                    if dc >= 1:
                        wo_mms(dc - 1, t_order[24:], 24)
                        drain(dc - 1)
                wo_mms(7, t_order[24:], 24)
                drain(7)
    nc.compile()
    return nc


# within each head: 4 windows of 32 partitions = [16 re-pairs | 16 im-pairs],
# so the rotate-half is stream_shuffle's per-32-window rotation by 16.
_PERM_IDX = np.array(
    [
        2 * (w * 16 + (j if j < 16 else j - 16)) + (0 if j < 16 else 1)
        for w in range(4)
        for j in range(32)
    ]
)
_PI = np.array([w * 16 + (j if j < 16 else j - 16) for w in range(4) for j in range(32)])
_SGN = np.array(
    [(-1.0 if j < 16 else 1.0) for w in range(4) for j in range(32)], np.float32
)


def _perm_block(w):
    o = w.reshape(-1, HD, D)
    return o[:, _PERM_IDX, :].reshape(-1, D)


def _stage(x, wq, wk, wv, wo, freqs_cos, freqs_sin, mask):
    """Returns (mode, shared dict, per-core dicts)."""
    causal = np.where(np.triu(np.ones((S, S), dtype=bool), k=1), -1e9, 0.0).astype(
        np.float32
    )
    if not mask.any():
        mode = "none"
    elif np.array_equal(mask, causal):
        mode = "causal"
    else:
        mode = "general"

    xT = np.ascontiguousarray(x.reshape(BS, D).T)  # [D, BS]
    x_st = (
        xT.reshape(NDT, 128, NSC, 512).transpose(2, 0, 1, 3).reshape(NSC * NDT * 128, 512)
    ).astype(bf16)
    woT = np.ascontiguousarray(wo.T)  # [hd, Dout]
    wo_st = (
        woT.reshape(NDT, 128, 8, 512).transpose(2, 0, 1, 3).reshape(8 * NDT * 128, 512)
    ).astype(bf16)
    cosT = freqs_cos.T.astype(np.float32)  # [64, S]
    sinT = freqs_sin.T.astype(np.float32)
    cos_st = np.ascontiguousarray(cosT[_PI, :])
    sin_st = np.ascontiguousarray(sinT[_PI, :] * _SGN[:, None])

    shared = {"xT": x_st, "coss": cos_st, "sins": sin_st, "wos": wo_st,
              "ident": np.eye(128, dtype=bf16)}
    if mode == "causal":
        # additive mask for the diagonal 512x512 block in [k, q] layout:
        # -1e9 where k > q (future), accumulated into the scores psum
        add_diag = np.where(
            np.tril(np.ones((512, 512), dtype=bool), -1), -1e9, 0.0
        ).astype(np.float32).astype(bf16)
        shared["binsA"] = np.ascontiguousarray(
            add_diag.reshape(4, 128, 512).transpose(1, 0, 2).reshape(128, 2048)
        )
    elif mode == "general":
        shared["maskT"] = np.ascontiguousarray(mask.T * math.sqrt(HD)).astype(bf16)

    per_core = []
    for c in range(NC):
        wq_c = _perm_block(wq[c * 512:(c + 1) * 512]).T  # [D, 512]
        wk_c = _perm_block(wk[c * 128:(c + 1) * 128]).T  # [D, 128]
        wv_c = wv[c * 128:(c + 1) * 128].T               # [D, 128]
        wq_st = wq_c.reshape(NDT, 128, 512).transpose(1, 0, 2).reshape(128, NDT * 512)
        wk_st = wk_c.reshape(NDT, 128, 128).transpose(1, 0, 2).reshape(128, NDT * 128)
        wv_st = wv_c.reshape(NDT, 128, 128).transpose(1, 0, 2).reshape(128, NDT * 128)
        per_core.append(
            {
                "wqs": np.ascontiguousarray(wq_st).astype(bf16),
                "wks": np.ascontiguousarray(wk_st).astype(bf16),
                "wvs": np.ascontiguousarray(wv_st).astype(bf16),
            }
        )
    return mode, shared, per_core


def _get_nc(mode):
    if mode not in _CACHE:
        _CACHE[mode] = _build(mode)
    return _CACHE[mode]


def kernel(x, wq, wk, wv, wo, freqs_cos, freqs_sin, mask, start_pos=0, **_kw):
    x = np.asarray(x, np.float32)
    wq = np.asarray(wq, np.float32)
    wk = np.asarray(wk, np.float32)
    wv = np.asarray(wv, np.float32)
    wo = np.asarray(wo, np.float32)
    freqs_cos = np.asarray(freqs_cos, np.float32)
    freqs_sin = np.asarray(freqs_sin, np.float32)
    mask = np.asarray(mask, np.float32)

    mode, shared, per_core = _stage(x, wq, wk, wv, wo, freqs_cos, freqs_sin, mask)
    nc = _get_nc(mode)
    in_maps = [dict(shared, **per_core[c]) for c in range(NC)]
    res = run_bass_kernel_spmd(nc, in_maps, core_ids=list(range(NC)))
    outs = [np.asarray(r["out"], np.float32) for r in res.results]
    return np.concatenate(outs, axis=0).reshape(B, S, D)
